# revision 9
# baseline (speedup 1.0000x reference)
"""Trainium2 Bass kernel for nn_MultiHeadAttention_78237124264578.

Reference computation (NO softmax — attention is purely bilinear):
    q = (x @ Wq.T + bq).reshape(8, 2, 2048, 64)   # FLAT reshape
    att = einsum('hbid,hbjd->hbij', q, k) * 64**-0.5
    out = einsum('hbij,hbjd->hbid', att, v)
    return out.transpose(1,2,3,0).reshape(2, 2048, 512)

Key identities exploited:
  1. (q kT) v == q (kT v): the 2048x2048 attention matrix collapses to a
     64x64 Gram matrix S = K^T V per (head, block b2).
  2. The head reshape is flat: head h / block b2 of Q/K/V is rows
     [512h + 256 b2, 512h + 256(b2+1)) of the [4096, 512] projection
     output, reinterpreted [256,512]->[2048,64].  So core i only needs
     x rows [512i, 512(i+1)) plus the full (512x512) weights.
  3. O_chunk[256r, 512f] = Yq_chunk @ (I8 (x) S): per 128-col chunk c the
     transposed output OT[128, 512r] = blockdiag(S, S)^T @ YqT_chunk, so
     one 128-partition matmul per (b2, chunk) computes O.

Sharding: head i -> core i.  All inputs shipped bf16 (halves the DMA,
matmul rate identical to fp32r, and small-free-size matmuls avoid the
fp32r 4x penalty).  The 0.125 attention scale is folded into Wq/bq on
the host.  Inputs are packed into ONE dram tensor in consumption order
so 8 pipelined DMAs feed the PE without stalls:
    slots: xt0 wk0 xt1 wk1 xt2 wk2 xt3 wk3 | wv0..3 | wq0..3
Stores go straight from PSUM to DRAM (no SBUF bounce), one [128,512]
DMA per output chunk.
"""

import functools

import numpy as np

NCORES = 8
NIN = 512          # input features = contraction dim
NF = 512           # projection output features
R = 512            # rows per core (one head)
KC = NIN // 128    # contraction chunks
FC = NF // 128     # feature/row chunks
DIM = 64
SCALE = DIM ** -0.5


@functools.lru_cache(maxsize=1)
def _build():
    from concourse import bacc
    import concourse.mybir as mybir
    import concourse.tile as tile

    f32 = mybir.dt.float32
    bf16 = mybir.dt.bfloat16

    nc = bacc.Bacc(None, target_bir_lowering=False)

    # packed operands: 16 slots of [128, 512] bf16, consumption order
    inp_d = nc.dram_tensor("inp", [128, 16, 512], bf16, kind="ExternalInput")
    brow_d = nc.dram_tensor("brow", [1, 2 * NF], f32, kind="ExternalInput")  # bk|bv
    bqc_d = nc.dram_tensor("bqc", [128, FC], f32, kind="ExternalInput")  # 0.125*bq
    ot_d = nc.dram_tensor("ot", [NF, R], bf16, kind="ExternalOutput")

    XT = [0, 2, 4, 6]   # xt k-chunk slots
    WK = [1, 3, 5, 7]
    WV = [8, 9, 10, 11]
    WQ = [12, 13, 14, 15]

    with tile.TileContext(nc) as tc:
        with (
            tc.tile_pool(name="sb", bufs=1) as sb,
            tc.tile_pool(name="pa", bufs=4, space="PSUM") as pa,
            tc.tile_pool(name="pb", bufs=4, space="PSUM") as pb,
        ):
            # No PE warm-up: the p-state ramp reference is the END of the
            # last busy stretch, so the t~71 init drain starts the 3us ramp
            # and the first real matmul (~3.6us) already runs at full clock.

            # ---- input DMAs: 8 x 2 slots via HWDGE, consumption order ------
            ops = sb.tile([128, 16, 512], bf16, tag="ops", name="ops")
            for t in range(8):
                nc.sync.dma_start(
                    ops[:, 2 * t:2 * t + 2, :], inp_d[:, 2 * t:2 * t + 2, :])

            brow = sb.tile([1, 2 * NF], f32, tag="brow")
            bqc = sb.tile([128, FC], f32, tag="bqc")
            bkb = sb.tile([128, NF], f32, tag="bkb")
            bvb = sb.tile([128, NF], f32, tag="bvb")
            nc.gpsimd.dma_start(brow[:], brow_d[:, :])
            nc.gpsimd.dma_start(bqc[:], bqc_d[:, :])
            nc.gpsimd.partition_broadcast(bkb[:], brow[0:1, 0:NF])
            nc.gpsimd.partition_broadcast(bvb[:], brow[0:1, NF:2 * NF])

            # blockdiag(S,S) operand tiles (off-diagonal stays zero)
            s2b = [sb.tile([128, 128], bf16, tag=f"s2b{b}", name=f"s2b{b}")
                   for b in range(2)]
            nc.gpsimd.memset(s2b[0][:], 0.0)
            nc.gpsimd.memset(s2b[1][:], 0.0)

            k_sb = [sb.tile([128, NF], bf16, tag=f"k{c}", name=f"k{c}") for c in range(FC)]
            v_sb = [sb.tile([128, NF], bf16, tag=f"v{c}", name=f"v{c}") for c in range(FC)]
            q_sb = [sb.tile([128, R], bf16, tag=f"q{c}", name=f"q{c}") for c in range(FC)]

            def slot(s):
                return ops[:, s, :]

            def slotc(s, c):
                return ops[:, s, 128 * c:128 * (c + 1)]

            # ---- Yk: psk[c][r,f], k-outer to match DMA arrival -------------
            psk = [pa.tile([128, NF], f32, tag="A", name=f"psk{c}") for c in range(FC)]
            for k in range(KC):
                for c in range(FC):
                    nc.tensor.matmul(
                        psk[c][:], slotc(XT[k], c), slot(WK[k]),
                        start=(k == 0), stop=(k == KC - 1),
                    )
            # K bias adds: PSUM tensor+tensor is DVE-only; c0/c1 first (S0)
            nc.vector.tensor_add(k_sb[0][:], psk[0][:], bkb[:])
            nc.vector.tensor_add(k_sb[1][:], psk[1][:], bkb[:])

            psv = [pb.tile([128, NF], f32, tag="B", name=f"psv{c}") for c in range(FC)]

            def yv(c):
                for k in range(KC):
                    nc.tensor.matmul(
                        psv[c][:], slotc(XT[k], c), slot(WV[k]),
                        start=(k == 0), stop=(k == KC - 1),
                    )

            def vbias(c):
                nc.vector.tensor_add(v_sb[c][:], psv[c][:], bvb[:])

            psq = [pa.tile([128, R], f32, tag="A", name=f"psq{c}") for c in range(FC)]

            def yq(c, half=None):
                sl = slice(None) if half is None else slice(256 * half, 256 * (half + 1))
                for k in range(KC):
                    nc.tensor.matmul(
                        psq[c][:, sl], slotc(WQ[k], c), slot(XT[k])[:, sl],
                        start=(k == 0), stop=(k == KC - 1),
                    )

            def s_mm(ps_s, b2):
                idx = 0
                for rc in (2 * b2, 2 * b2 + 1):
                    for fh in range(8):
                        nc.tensor.matmul(
                            ps_s[:],
                            k_sb[rc][:, 64 * fh:64 * (fh + 1)],
                            v_sb[rc][:, 64 * fh:64 * (fh + 1)],
                            start=(idx == 0), stop=(idx == 15),
                        )
                        idx += 1

            # ---- pipeline: Yv c0/c1 -> Yq c0 -> S0 -> Yv c2/c3 -> Yq c1 ->
            #      S1 -> Yq c2 -> O pairs + stores -> Yq c3 (split) ---------
            def qbias_act(c, sl=slice(None)):
                nc.scalar.activation(
                    q_sb[c][:, sl], psq[c][:, sl],
                    mybir.ActivationFunctionType.Identity,
                    bias=bqc[:, c:c + 1], scale=1.0,
                )

            yv(0)
            yv(1)
            vbias(0)
            vbias(1)
            yq(0)
            qbias_act(0)
            ps_s0 = pb.tile([64, 64], f32, tag="B", name="ps_s0")
            s_mm(ps_s0, 0)
            # K c2/c3 adds queue on DVE behind v0/v1; blockdiag halves
            # split DVE/ACT for latency
            nc.vector.tensor_add(k_sb[2][:], psk[2][:], bkb[:])
            nc.vector.tensor_add(k_sb[3][:], psk[3][:], bkb[:])
            nc.vector.tensor_copy(s2b[0][0:64, 0:64], ps_s0[:])
            nc.scalar.copy(s2b[0][64:128, 64:128], ps_s0[:])
            yv(2)
            vbias(2)
            yv(3)
            vbias(3)
            yq(1)
            qbias_act(1)
            ps_s1 = pb.tile([64, 64], f32, tag="B", name="ps_s1")
            s_mm(ps_s1, 1)
            nc.vector.tensor_copy(s2b[1][0:64, 0:64], ps_s1[:])
            nc.scalar.copy(s2b[1][64:128, 64:128], ps_s1[:])
            yq(2)
            qbias_act(2)

            # output chunks: ps_oc[c] [128, 512], col half b2 from s2b[b2]
            ps_oc = [None] * FC
            ps_oc[0] = pb.tile([128, R], f32, tag="B", name="ps_oc0")
            ps_oc[1] = pb.tile([128, R], f32, tag="B", name="ps_oc1")
            ps_oc[2] = pa.tile([128, R], f32, tag="A", name="ps_oc2")
            ps_oc[3] = pa.tile([128, R], f32, tag="A", name="ps_oc3")

            oc_sb = [sb.tile([128, R], bf16, tag=f"oc{c}", name=f"oc{c}")
                     for c in range(FC)]

            def o_pair(c):
                for b2 in range(2):
                    rsl = slice(256 * b2, 256 * (b2 + 1))
                    nc.tensor.matmul(ps_oc[c][:, rsl], s2b[b2][:], q_sb[c][:, rsl])
                # PSUM -> SBUF bf16, halves split DVE/ACT for latency
                nc.vector.tensor_copy(oc_sb[c][:, 0:256], ps_oc[c][:, 0:256])
                nc.scalar.copy(oc_sb[c][:, 256:512], ps_oc[c][:, 256:512])
                nc.sync.dma_start(ot_d[128 * c:128 * (c + 1), :], oc_sb[c][:])

            o_pair(0)
            o_pair(1)
            yq(3, half=0)
            nc.vector.tensor_scalar_add(
                q_sb[3][:, 0:256], psq[3][:, 0:256], bqc[:, 3:4])
            yq(3, half=1)
            qbias_act(3, sl=slice(256, 512))
            o_pair(2)
            o_pair(3)

    nc.compile()
    return nc


def kernel(x, Wq, bq, Wk, bk, Wv, bv):
    import ml_dtypes
    from concourse.bass_utils import run_bass_kernel_spmd

    bf16 = ml_dtypes.bfloat16
    x = np.asarray(x, dtype=np.float32)
    Wq = np.asarray(Wq, dtype=np.float32)
    Wk = np.asarray(Wk, dtype=np.float32)
    Wv = np.asarray(Wv, dtype=np.float32)
    bq = np.asarray(bq, dtype=np.float32)
    bk = np.asarray(bk, dtype=np.float32)
    bv = np.asarray(bv, dtype=np.float32)

    B, N, nin = x.shape
    x_flat = x.reshape(B * N, nin)                       # [4096, 512]

    wkt = Wk.T.astype(bf16)                              # [k, f]
    wvt = Wv.T.astype(bf16)
    wqt = (SCALE * Wq).T.astype(bf16)
    brow = np.ascontiguousarray(
        np.concatenate([bk, bv]).reshape(1, 2 * NF))
    bqc = np.ascontiguousarray((SCALE * bq).reshape(FC, 128).T)  # [p, c]

    def chunks(t):
        return [t[128 * j:128 * (j + 1)] for j in range(4)]

    wk_c, wv_c, wq_c = chunks(wkt), chunks(wvt), chunks(wqt)

    in_maps = []
    for i in range(NCORES):
        xt_i = x_flat[R * i:R * (i + 1)].T.astype(bf16)  # [k, r]
        xt_c = chunks(xt_i)
        slots = [xt_c[0], wk_c[0], xt_c[1], wk_c[1],
                 xt_c[2], wk_c[2], xt_c[3], wk_c[3],
                 *wv_c, *wq_c]
        inp = np.ascontiguousarray(np.stack(slots, axis=1))  # [128, 16, 512]
        in_maps.append({"inp": inp, "brow": brow, "bqc": bqc})

    nc = _build()
    res = run_bass_kernel_spmd(nc, in_maps, core_ids=list(range(NCORES)))

    # ot[i][f_hi*64+d, b2*256+rr] = out[h=i, b2, n2=rr*8+f_hi, d]
    ot = np.stack([np.asarray(res.results[i]["ot"], dtype=np.float32)
                   for i in range(NCORES)])                       # [h, f', r]
    ot = ot.reshape(NCORES, 8, DIM, 2, 256)                       # [h, fh, d, b2, rr]
    z = ot.transpose(3, 4, 1, 2, 0).reshape(B, N, 8 * DIM)        # [b2, n2, d*8+h]
    return np.ascontiguousarray(z)


# revision 11
# speedup vs baseline: 1.2458x; 1.2458x over previous
"""Trainium2 Bass kernel for nn_MultiHeadAttention_78237124264578.

Reference computation (NO softmax — attention is purely bilinear):
    q = (x @ Wq.T + bq).reshape(8, 2, 2048, 64)   # FLAT reshape
    att = einsum('hbid,hbjd->hbij', q, k) * 64**-0.5
    out = einsum('hbij,hbjd->hbid', att, v)
    return out.transpose(1,2,3,0).reshape(2, 2048, 512)

Key identities exploited:
  1. (q kT) v == q (kT v): the 2048x2048 attention matrix collapses to a
     64x64 Gram matrix S = K^T V per (head, block b2).
  2. The head reshape is flat: head h / block b2 of Q/K/V is rows
     [512h + 256 b2, 512h + 256(b2+1)) of the [4096, 512] projection
     output, reinterpreted [256,512]->[2048,64].  So core i only needs
     x rows [512i, 512(i+1)) plus the full (512x512) weights.
  3. O_chunk[256r, 512f] = Yq_chunk @ (I8 (x) S): per 128-col chunk c the
     transposed output OT[128, 512r] = blockdiag(S, S)^T @ YqT_chunk, so
     one 128-partition matmul per (b2, chunk) computes O.

Sharding: head i -> core i.  All inputs shipped bf16 (halves the DMA,
matmul rate identical to fp32r, and small-free-size matmuls avoid the
fp32r 4x penalty).  The 0.125 attention scale is folded into Wq/bq on
the host.  Inputs are packed into ONE dram tensor in consumption order
so 8 pipelined DMAs feed the PE without stalls:
    slots: xt0 wk0 xt1 wk1 xt2 wk2 xt3 wk3 | wv0..3 | wq0..3
Stores go straight from PSUM to DRAM (no SBUF bounce), one [128,512]
DMA per output chunk.
"""

import functools

import numpy as np

NCORES = 8
NIN = 512          # input features = contraction dim
NF = 512           # projection output features
R = 512            # rows per core (one head)
KC = NIN // 128    # contraction chunks
FC = NF // 128     # feature/row chunks
DIM = 64
SCALE = DIM ** -0.5

# PE warm-up matmuls issued before the real work (ramps the PE clock
# while the first input DMAs are in flight).
N_WARMUP = 6


@functools.lru_cache(maxsize=1)
def _build():
    from concourse import bacc
    import concourse.mybir as mybir
    import concourse.tile as tile

    f32 = mybir.dt.float32
    bf16 = mybir.dt.bfloat16

    nc = bacc.Bacc(None, target_bir_lowering=False)

    # packed operands: 16 slots of [128, 512] bf16, consumption order
    inp_d = nc.dram_tensor("inp", [128, 16, 512], bf16, kind="ExternalInput")
    brow_d = nc.dram_tensor("brow", [1, 2 * NF], f32, kind="ExternalInput")  # bk|bv
    bqc_d = nc.dram_tensor("bqc", [128, FC], f32, kind="ExternalInput")  # 0.125*bq
    ot_d = nc.dram_tensor("ot", [NF, R], bf16, kind="ExternalOutput")

    XT = [0, 2, 4, 6]   # xt k-chunk slots
    WK = [1, 3, 5, 7]
    WV = [8, 9, 10, 11]
    WQ = [12, 13, 14, 15]

    with tile.TileContext(nc) as tc:
        with (
            tc.tile_pool(name="sb", bufs=1) as sb,
            tc.tile_pool(name="pa", bufs=4, space="PSUM") as pa,
            tc.tile_pool(name="pb", bufs=4, space="PSUM") as pb,
        ):
            # PE warm-up matmuls: keep the tensor engine's p-state ramping
            # while the first input DMAs are in flight (without these the
            # cost model drops mid-stream matmuls to the low p-state).
            wu = sb.tile([1, 128], f32, tag="wu", name="wu")
            nc.vector.memset(wu[:], 0.0)
            for i in range(N_WARMUP):
                psw = pb.tile([1, 128], f32, tag="B", name=f"psw{i}")
                nc.tensor.matmul(psw[:], wu[0:1, 0:1], wu[:])

            # ---- input DMAs: 8 x 2 slots via HWDGE, consumption order ------
            ops = sb.tile([128, 16, 512], bf16, tag="ops", name="ops")
            for t in range(8):
                nc.sync.dma_start(
                    ops[:, 2 * t:2 * t + 2, :], inp_d[:, 2 * t:2 * t + 2, :])

            brow = sb.tile([1, 2 * NF], f32, tag="brow")
            bqc = sb.tile([128, FC], f32, tag="bqc")
            bkb = sb.tile([128, NF], f32, tag="bkb")
            bvb = sb.tile([128, NF], f32, tag="bvb")
            nc.gpsimd.dma_start(brow[:], brow_d[:, :])
            nc.gpsimd.dma_start(bqc[:], bqc_d[:, :])
            nc.gpsimd.partition_broadcast(bkb[:], brow[0:1, 0:NF])
            nc.gpsimd.partition_broadcast(bvb[:], brow[0:1, NF:2 * NF])

            # blockdiag(S,S) operand tiles (off-diagonal stays zero)
            s2b = [sb.tile([128, 128], bf16, tag=f"s2b{b}", name=f"s2b{b}")
                   for b in range(2)]
            nc.gpsimd.memset(s2b[0][:], 0.0)
            nc.gpsimd.memset(s2b[1][:], 0.0)

            k_sb = [sb.tile([128, NF], bf16, tag=f"k{c}", name=f"k{c}") for c in range(FC)]
            v_sb = [sb.tile([128, NF], bf16, tag=f"v{c}", name=f"v{c}") for c in range(FC)]
            q_sb = [sb.tile([128, R], bf16, tag=f"q{c}", name=f"q{c}") for c in range(FC)]

            def slot(s):
                return ops[:, s, :]

            def slotc(s, c):
                return ops[:, s, 128 * c:128 * (c + 1)]

            # ---- Yk: psk[c][r,f], k-outer to match DMA arrival -------------
            psk = [pa.tile([128, NF], f32, tag="A", name=f"psk{c}") for c in range(FC)]
            for k in range(KC):
                for c in range(FC):
                    nc.tensor.matmul(
                        psk[c][:], slotc(XT[k], c), slot(WK[k]),
                        start=(k == 0), stop=(k == KC - 1),
                    )
            # K bias adds: PSUM tensor+tensor is DVE-only; c0/c1 first (S0)
            nc.vector.tensor_add(k_sb[0][:], psk[0][:], bkb[:])
            nc.vector.tensor_add(k_sb[1][:], psk[1][:], bkb[:])

            psv = [pb.tile([128, NF], f32, tag="B", name=f"psv{c}") for c in range(FC)]

            def yv(c):
                for k in range(KC):
                    nc.tensor.matmul(
                        psv[c][:], slotc(XT[k], c), slot(WV[k]),
                        start=(k == 0), stop=(k == KC - 1),
                    )

            def vbias(c):
                nc.vector.tensor_add(v_sb[c][:], psv[c][:], bvb[:])

            psq = [pa.tile([128, R], f32, tag="A", name=f"psq{c}") for c in range(FC)]

            def yq(c, half=None):
                sl = slice(None) if half is None else slice(256 * half, 256 * (half + 1))
                for k in range(KC):
                    nc.tensor.matmul(
                        psq[c][:, sl], slotc(WQ[k], c), slot(XT[k])[:, sl],
                        start=(k == 0), stop=(k == KC - 1),
                    )

            def s_mm(ps_s, b2):
                idx = 0
                for rc in (2 * b2, 2 * b2 + 1):
                    for fh in range(8):
                        nc.tensor.matmul(
                            ps_s[:],
                            k_sb[rc][:, 64 * fh:64 * (fh + 1)],
                            v_sb[rc][:, 64 * fh:64 * (fh + 1)],
                            start=(idx == 0), stop=(idx == 15),
                        )
                        idx += 1

            # ---- pipeline: Yv c0/c1 -> Yq c0 -> S0 -> Yv c2/c3 -> Yq c1 ->
            #      S1 -> Yq c2 -> O pairs + stores -> Yq c3 (split) ---------
            def qbias_act(c, sl=slice(None)):
                nc.scalar.activation(
                    q_sb[c][:, sl], psq[c][:, sl],
                    mybir.ActivationFunctionType.Identity,
                    bias=bqc[:, c:c + 1], scale=1.0,
                )

            yv(0)
            yv(1)
            vbias(0)
            vbias(1)
            yq(0)
            qbias_act(0)
            ps_s0 = pb.tile([64, 64], f32, tag="B", name="ps_s0")
            s_mm(ps_s0, 0)
            # K c2/c3 adds queue on DVE behind v0/v1; blockdiag halves
            # split DVE/ACT for latency
            nc.vector.tensor_add(k_sb[2][:], psk[2][:], bkb[:])
            nc.vector.tensor_add(k_sb[3][:], psk[3][:], bkb[:])
            nc.vector.tensor_copy(s2b[0][0:64, 0:64], ps_s0[:])
            nc.scalar.copy(s2b[0][64:128, 64:128], ps_s0[:])
            yv(2)
            vbias(2)
            yv(3)
            vbias(3)
            yq(1)
            qbias_act(1)
            ps_s1 = pb.tile([64, 64], f32, tag="B", name="ps_s1")
            s_mm(ps_s1, 1)
            nc.vector.tensor_copy(s2b[1][0:64, 0:64], ps_s1[:])
            nc.scalar.copy(s2b[1][64:128, 64:128], ps_s1[:])
            yq(2)
            qbias_act(2)

            # output chunks: ps_oc[c] [128, 512], col half b2 from s2b[b2]
            ps_oc = [None] * FC
            ps_oc[0] = pb.tile([128, R], f32, tag="B", name="ps_oc0")
            ps_oc[1] = pb.tile([128, R], f32, tag="B", name="ps_oc1")
            ps_oc[2] = pa.tile([128, R], f32, tag="A", name="ps_oc2")
            ps_oc[3] = pa.tile([128, R], f32, tag="A", name="ps_oc3")

            oc_sb = [sb.tile([128, R], bf16, tag=f"oc{c}", name=f"oc{c}")
                     for c in range(FC)]

            def o_pair(c):
                for b2 in range(2):
                    rsl = slice(256 * b2, 256 * (b2 + 1))
                    nc.tensor.matmul(ps_oc[c][:, rsl], s2b[b2][:], q_sb[c][:, rsl])
                # PSUM -> SBUF bf16, halves split DVE/ACT for latency
                nc.vector.tensor_copy(oc_sb[c][:, 0:256], ps_oc[c][:, 0:256])
                nc.scalar.copy(oc_sb[c][:, 256:512], ps_oc[c][:, 256:512])
                nc.sync.dma_start(ot_d[128 * c:128 * (c + 1), :], oc_sb[c][:])

            o_pair(0)
            o_pair(1)
            yq(3, half=0)
            nc.vector.tensor_scalar_add(
                q_sb[3][:, 0:256], psq[3][:, 0:256], bqc[:, 3:4])
            yq(3, half=1)
            qbias_act(3, sl=slice(256, 512))
            o_pair(2)
            o_pair(3)

    nc.compile()
    return nc


def kernel(x, Wq, bq, Wk, bk, Wv, bv):
    import ml_dtypes
    from concourse.bass_utils import run_bass_kernel_spmd

    bf16 = ml_dtypes.bfloat16
    x = np.asarray(x, dtype=np.float32)
    Wq = np.asarray(Wq, dtype=np.float32)
    Wk = np.asarray(Wk, dtype=np.float32)
    Wv = np.asarray(Wv, dtype=np.float32)
    bq = np.asarray(bq, dtype=np.float32)
    bk = np.asarray(bk, dtype=np.float32)
    bv = np.asarray(bv, dtype=np.float32)

    B, N, nin = x.shape
    x_flat = x.reshape(B * N, nin)                       # [4096, 512]

    wkt = Wk.T.astype(bf16)                              # [k, f]
    wvt = Wv.T.astype(bf16)
    wqt = (SCALE * Wq).T.astype(bf16)
    brow = np.ascontiguousarray(
        np.concatenate([bk, bv]).reshape(1, 2 * NF))
    bqc = np.ascontiguousarray((SCALE * bq).reshape(FC, 128).T)  # [p, c]

    def chunks(t):
        return [t[128 * j:128 * (j + 1)] for j in range(4)]

    wk_c, wv_c, wq_c = chunks(wkt), chunks(wvt), chunks(wqt)

    in_maps = []
    for i in range(NCORES):
        xt_i = x_flat[R * i:R * (i + 1)].T.astype(bf16)  # [k, r]
        xt_c = chunks(xt_i)
        slots = [xt_c[0], wk_c[0], xt_c[1], wk_c[1],
                 xt_c[2], wk_c[2], xt_c[3], wk_c[3],
                 *wv_c, *wq_c]
        inp = np.ascontiguousarray(np.stack(slots, axis=1))  # [128, 16, 512]
        in_maps.append({"inp": inp, "brow": brow, "bqc": bqc})

    nc = _build()
    res = run_bass_kernel_spmd(nc, in_maps, core_ids=list(range(NCORES)))

    # ot[i][f_hi*64+d, b2*256+rr] = out[h=i, b2, n2=rr*8+f_hi, d]
    ot = np.stack([np.asarray(res.results[i]["ot"], dtype=np.float32)
                   for i in range(NCORES)])                       # [h, f', r]
    ot = ot.reshape(NCORES, 8, DIM, 2, 256)                       # [h, fh, d, b2, rr]
    z = ot.transpose(3, 4, 1, 2, 0).reshape(B, N, 8 * DIM)        # [b2, n2, d*8+h]
    return np.ascontiguousarray(z)
